# revision 41
# baseline (speedup 1.0000x reference)
"""Trainium2 Bass kernel: MoE-routed ARIMAX forecast combine.

Computes, for full inputs (B=8192 rows):
  psi   = softmax_k(-d2),  d2[b,k] = (mu_k - z_b)^T S_k (mu_k - z_b),  S_k = sig_k sig_k^T
  preds = K-expert linear AR(8) recursion rolled out T_OUT=24 steps
  y_con = sum_k psi[b,k] * preds[b,k,:]
  x_recon = zeros

Strategy:
  * Data-parallel over batch across 8 NeuronCores (1024 rows/core); the small
    parameters are replicated.
  * The AR recursion is linear in (y8, ex), so it collapses into transfer
    matrices G[K,T,O], H[K,T] precomputed on host from A (tiny, O(K*T*O)).
  * When sigma_inv is identical across clusters (true for this model: it is a
    shared covariance), z^T S z is cluster-independent and cancels in the
    row-softmax, so d2 reduces (up to a per-row additive shift) to
    c_k - 2 z.(S mu_k), which is a [B,128]x[128,32] matmul. This is also
    ~100x closer to the exact result than the reference's own f32 rounding.
  * If sigma_inv ever differed per cluster, we fall back to a host computation
    of the same math (correct for arbitrary inputs).

Device structure per core (1024 rows = 8 tiles of 128 partitions), processed
as two pipelined 4-tile halves:
  - Input DMAs split across both HWDGE queues (sync/scalar) + gpsimd SWDGE,
    sized/ordered for DMA latency: big contiguous per-partition runs
    (descriptor size), small first-wave operands first, 16-row u^T (the
    exogenous bias folds into the combine matmul as psi @ (H*bias), which
    also removes the ones-row that made the u^T DMA 17 rows/slow).
  - PE per tile: d2' = -2 z.(S mu_k) and ex = u@Bx^T as independent
    single-group matmuls into two separate PSUM banks (the softmax DVE
    reads would otherwise serialize behind the ex matmuls; and a start=True
    marks its whole 2KB bank pending-zero, so cross-matmul PSUM
    accumulation in a shared bank is avoided entirely).
  - Softmax over k batched across the half's 4 tiles with strided APs:
    +c_k (host-replicated const) -> rowmin -> sub -> exp(ACT) -> rowsum ->
    1/s -> px = e*ex  (psi deferred: 1/s is applied to the final y).
  - PE: 2 transposes of [psi|px] tile pairs; per-tile fused combine matmul
    with block-diagonal [G2 0; 0 H] -> [P | yh]; consecutive tiles use
    different PE row-groups and run concurrently, so each tile's output
    gets its own PSUM bank (concurrent row-groups writing one PSUM bank
    crash the exec unit).
  - DVE per half: y = (sum_o P[:,t,o]*y8[:,o] + yh) * r, with P t-major so
    the o-reduction reads contiguously; output DMA per half.
"""

import numpy as np

B = 8192
K = 32
D = 128
ORDER = 8
T_OUT = 24
E = 16
IN_LEN = 96
N_CORES = 8
BPC = B // N_CORES       # rows per core
TILE = 128
NT = BPC // TILE         # tiles per core
MIN_CLAMP = 0.0
OT = ORDER * T_OUT       # 192
OT1 = (ORDER + 1) * T_OUT  # 216

# packed const layout (columns of the [128, 440] const tensor); the first 96
# columns are the only ones needed by the psa matmul + softmax stages
C_W2, C_RHSA, C_CREP, C_ID, C_G2H = 0, 32, 64, 96, 224
CW = 440

_PROGRAM = None

# test.py can flip these to get a profile out of the run
TRACE = False
LAST_RESULTS = None


def _build_program():
    import concourse.bacc as bacc
    import concourse.mybir as mybir
    import concourse.tile as tile
    from contextlib import ExitStack

    F32 = mybir.dt.float32
    AX = mybir.AxisListType
    OP = mybir.AluOpType
    AF = mybir.ActivationFunctionType

    nc = bacc.Bacc("TRN2", target_bir_lowering=False, debug=False)

    zT = nc.dram_tensor("zT", [D, BPC], F32, kind="ExternalInput")
    uTa = nc.dram_tensor("uTa", [E, BPC], F32, kind="ExternalInput")
    y8d = nc.dram_tensor("y8", [TILE, NT * ORDER], F32, kind="ExternalInput")
    cst = nc.dram_tensor("cst", [TILE, CW], F32, kind="ExternalInput")
    ycon = nc.dram_tensor("ycon", [TILE, NT * T_OUT], F32, kind="ExternalOutput")

    with tile.TileContext(nc) as tc, ExitStack() as ctx:
        sb = ctx.enter_context(tc.tile_pool(name="sb", bufs=1))
        wk = ctx.enter_context(tc.tile_pool(name="wk", bufs=2))
        pst_pool = ctx.enter_context(tc.tile_pool(name="pst", bufs=1, space="PSUM"))
        psa_pool = ctx.enter_context(tc.tile_pool(name="psa", bufs=1, space="PSUM"))
        pspy_pool = ctx.enter_context(tc.tile_pool(name="pspy", bufs=1, space="PSUM"))

        # DMAs split across both HWDGE queues (sync, scalar) + gpsimd SWDGE,
        # ordered so the first half's operands land first.
        cst_s = sb.tile([TILE, CW], F32)
        zT_s = sb.tile([D, BPC], F32)
        uTa_s = sb.tile([E, BPC], F32)
        y8_s = sb.tile([TILE, NT * ORDER], F32)
        HB = BPC // 2
        nc.sync.dma_start(zT_s[:, 0:HB], zT.ap()[:, 0:HB])
        nc.scalar.dma_start(cst_s[:, 0:C_ID], cst.ap()[:, 0:C_ID])
        nc.sync.dma_start(zT_s[:, HB:BPC], zT.ap()[:, HB:BPC])
        nc.scalar.dma_start(uTa_s[:], uTa.ap())
        nc.scalar.dma_start(cst_s[:, C_ID:CW], cst.ap()[:, C_ID:CW])
        nc.gpsimd.dma_start(y8_s[:], y8d.ap())

        w2m2 = cst_s[:, C_W2:C_W2 + K]
        rhsa = cst_s[0:E, C_RHSA:C_RHSA + K]
        ident = cst_s[:, C_ID:C_ID + TILE]
        crep = cst_s[:, C_CREP:C_CREP + K]

        # psd holds -2 z.(S mu), pse holds ex -- separate PSUM banks so the
        # softmax's DVE reads of psd aren't serialized behind the uTa matmuls
        # (Tile's bank tracker orders same-bank PE-writes vs DVE-reads).
        # Every matmul is a self-contained start+stop group (a start=True
        # marks its whole 2KB bank pending-zero, so cross-matmul accumulation
        # in a shared bank only works strictly interleaved -- avoided; the c_k
        # bias is added on DVE from a host-replicated constant instead)
        # psd split per half into separate banks: otherwise half1's z matmuls
        # and half0's softmax reads share a bank and the tracker serializes
        # them, forcing all z matmuls ahead of the first softmax op
        psd0 = psa_pool.tile([TILE, 4 * K], F32, tag="psd0")
        psd1 = psa_pool.tile([TILE, 4 * K], F32, tag="psd1")
        psd01 = [psd0, psd1]
        pse = psa_pool.tile([TILE, NT * K], F32, tag="pse")
        pspy = pspy_pool.tile([TILE, 4 * 512], F32)

        for h in range(2):
            # z matmuls first: the softmax min/exp chain depends only on them
            # (ex enters later via px), so it can overlap the uTa matmuls
            psd = psd01[h]
            for i in range(4 * h, 4 * h + 4):
                q = i - 4 * h
                nc.tensor.matmul(psd[:, 32 * q:32 * q + K],
                                 zT_s[:, i * TILE:(i + 1) * TILE], w2m2,
                                 start=True, stop=True)
            for i in range(4 * h, 4 * h + 4):
                nc.tensor.matmul(pse[:, 32 * i:32 * i + K],
                                 uTa_s[:, i * TILE:(i + 1) * TILE],
                                 rhsa, start=True, stop=True)

            d2v = psd[:].rearrange("p (n w) -> p n w", w=K)
            exv = pse[:, 128 * h:128 * (h + 1)].rearrange("p (n w) -> p n w", w=K)

            # softmax over k, batched across the half's 4 tiles
            t4 = wk.tile([TILE, 4 * K], F32, tag="t4")
            nc.vector.tensor_tensor(
                t4[:].rearrange("p (n w) -> p n w", w=K), d2v,
                crep.unsqueeze(1).broadcast_to([TILE, 4, K]), op=OP.add)
            t4v = t4[:].rearrange("p (n w) -> p n w", w=K)
            min4 = wk.tile([TILE, 4], F32, tag="min4")
            nc.vector.tensor_reduce(min4[:], t4v, axis=AX.X, op=OP.min)
            dn = wk.tile([TILE, 4 * K], F32, tag="dn")
            nc.vector.tensor_tensor(
                dn[:].rearrange("p (n w) -> p n w", w=K), t4v,
                min4[:].unsqueeze(2).broadcast_to([TILE, 4, K]), op=OP.subtract)
            # epx: per tile [e(32) | px(32)]
            epx = wk.tile([TILE, 4 * 2 * K], F32, tag="epx")
            ev = epx[:].rearrange("p (n w) -> p n w", w=2 * K)[:, :, 0:K]
            pxv = epx[:].rearrange("p (n w) -> p n w", w=2 * K)[:, :, K:2 * K]
            nc.scalar.activation(ev, dn[:].rearrange("p (n w) -> p n w", w=K),
                                 AF.Exp, scale=-1.0)
            s4 = wk.tile([TILE, 4], F32, tag="s4")
            nc.vector.tensor_reduce(s4[:], ev, axis=AX.X, op=OP.add)
            r4 = wk.tile([TILE, 4], F32, tag="r4")
            nc.vector.reciprocal(r4[:], s4[:])
            nc.vector.tensor_tensor(pxv, ev, exv, op=OP.mult)

            # transpose tile pairs: [128b, psi_i|px_i|psi_j|px_j] -> [128, 128b]
            epxT = wk.tile([TILE, 2 * TILE], F32, tag="epxT")
            for jl in range(2):
                pst = pst_pool.tile([TILE, TILE], F32, tag="pst")
                nc.tensor.transpose(pst[:], epx[:, jl * TILE:(jl + 1) * TILE], ident)
                nc.scalar.copy(epxT[:, jl * TILE:(jl + 1) * TILE], pst[:])

            # per-tile fused combine matmul: [P | yh] = [psi | px] @ [G2 0; 0 H]
            # consecutive tiles use different PE row-groups (base 0/64) and run
            # concurrently -> each gets its own PSUM bank; the second half
            # reuses the banks (WAR dep on this half's combine reads orders it)
            for q in range(4):
                i = 4 * h + q
                lhsT = epxT[64 * (i % 2):64 * (i % 2) + 64,
                            (q // 2) * TILE:(q // 2) * TILE + TILE]
                rhs = cst_s[64 * (i % 2):64 * (i % 2) + 64, C_G2H:C_G2H + OT1]
                nc.tensor.matmul(pspy[:, 512 * q:512 * q + OT1], lhsT, rhs,
                                 start=True, stop=True)

            # combine: y = (sum_o P[:,t,o]*y8[:,o] + yh) * r   (t-major P layout)
            # the last half is split into 2-tile chunks so its DVE chain and
            # output DMA start while the later combine matmuls still run
            for q0, nq in ([(0, 4)] if h == 0 else [(0, 2), (2, 2)]):
              pv = pspy[:].rearrange("p (n w) -> p n w", w=512)[:, q0:q0 + nq, 0:OT] \
                  .rearrange("p n (t o) -> p n t o", o=ORDER)
              y8v = y8_s[:].rearrange("p (n o) -> p n o", o=ORDER) \
                  [:, 4 * h + q0:4 * h + q0 + nq, :] \
                  .unsqueeze(2).broadcast_to([TILE, nq, T_OUT, ORDER])
              m4 = wk.tile([TILE, nq * OT], F32, tag="m4")
              nc.vector.tensor_tensor(
                  m4[:].rearrange("p (n t o) -> p n t o", o=ORDER, t=T_OUT), pv, y8v,
                  op=OP.mult)
              yg4 = wk.tile([TILE, nq * T_OUT], F32, tag="yg4")
              nc.vector.tensor_reduce(
                  yg4[:].rearrange("p (n t) -> p n t", t=T_OUT),
                  m4[:].rearrange("p (n t o) -> p n t o", o=ORDER, t=T_OUT),
                  axis=AX.X, op=OP.add)
              yhv = pspy[:].rearrange("p (n w) -> p n w", w=512)[:, q0:q0 + nq, OT:OT1]
              ya4 = wk.tile([TILE, nq * T_OUT], F32, tag="ya4")
              nc.vector.tensor_tensor(
                  ya4[:].rearrange("p (n t) -> p n t", t=T_OUT),
                  yg4[:].rearrange("p (n t) -> p n t", t=T_OUT), yhv, op=OP.add)
              rv = r4[:, q0:q0 + nq].unsqueeze(2).broadcast_to([TILE, nq, T_OUT])
              yo = wk.tile([TILE, nq * T_OUT], F32, tag="yo")
              nc.vector.tensor_tensor(
                  yo[:].rearrange("p (n t) -> p n t", t=T_OUT),
                  ya4[:].rearrange("p (n t) -> p n t", t=T_OUT), rv, op=OP.mult)
              eng = nc.sync if (h == 0 or q0 == 0) else nc.scalar
              c0 = (4 * h + q0) * T_OUT
              eng.dma_start(ycon.ap()[:, c0:c0 + nq * T_OUT], yo[:])

    nc.finalize()
    return nc


def _gh_from_A(A):
    """Transfer matrices of the linear AR recursion: preds[t] = G[k,t,:]@y8 + H[k,t]*ex."""
    A64 = A.astype(np.float64)
    G = np.zeros((K, T_OUT, ORDER))
    H = np.zeros((K, T_OUT))
    wy = np.broadcast_to(np.eye(ORDER), (K, ORDER, ORDER)).copy()
    we = np.zeros((K, ORDER))
    for t in range(T_OUT):
        gy = np.einsum('ko,koj->kj', A64, wy)
        ge = np.einsum('ko,ko->k', A64, we) + 1.0
        G[:, t, :] = gy
        H[:, t] = ge
        wy = np.concatenate([wy[:, 1:, :], gy[:, None, :]], axis=1)
        we = np.concatenate([we[:, 1:], ge[:, None]], axis=1)
    return G, H


def _prep_in_maps(y, z, u, mu, sigma_inv, A, Bx, bias):
    """Host parameter preprocessing + data layout; returns per-core input maps."""
    s0 = sigma_inv[0].astype(np.float64)
    S = s0 @ s0.T
    W = S @ mu.astype(np.float64).T                    # [D, K] = S mu_k
    c = np.einsum('kd,dk->k', mu.astype(np.float64), W)
    G, H = _gh_from_A(A)

    cst = np.zeros((TILE, CW), dtype=np.float32)
    cst[:, C_W2:C_W2 + K] = (-2.0 * W).astype(np.float32)
    cst[0:E, C_RHSA:C_RHSA + K] = Bx.T
    cst[:, C_CREP:C_CREP + K] = c.astype(np.float32)[None, :]
    cst[:, C_ID:C_ID + TILE] = np.eye(TILE, dtype=np.float32)
    # psi-block rhs [G2 | H*bias]: the exogenous bias is b-independent, so its
    # preds contribution psi @ (H*bias) folds into the psi-side combine matmul
    # (this removes the ones-row from uTa -> a 16-row DMA, 16-way SDMA split)
    g2h = np.zeros((K, OT1), dtype=np.float32)
    g2h[:, 0:OT] = G.reshape(K, OT)          # t-major: [k, t*ORDER + o]
    g2h[:, OT:OT1] = H * bias.astype(np.float64)[:, None]
    cst[0:K, C_G2H:C_G2H + OT1] = g2h
    cst[K:2 * K, C_G2H + OT:C_G2H + OT1] = H
    cst[64:64 + K, C_G2H:C_G2H + OT1] = g2h           # copy for base-64 row group
    cst[64 + K:64 + 2 * K, C_G2H + OT:C_G2H + OT1] = H

    zT = np.ascontiguousarray(z[:, 0, :].T)            # [D, B]
    uTa = np.ascontiguousarray(u.T)                    # [E, B]
    y8 = np.ascontiguousarray(y[:, IN_LEN - ORDER:])   # [B, 8]

    in_maps = []
    for cidx in range(N_CORES):
        lo, hi = cidx * BPC, (cidx + 1) * BPC
        y8c = y8[lo:hi].reshape(NT, TILE, ORDER).transpose(1, 0, 2)
        in_maps.append({
            "zT": np.ascontiguousarray(zT[:, lo:hi]),
            "uTa": np.ascontiguousarray(uTa[:, lo:hi]),
            "y8": np.ascontiguousarray(y8c.reshape(TILE, NT * ORDER)),
            "cst": cst,
        })
    return in_maps


def _host_fallback(y, z, u, mu, sigma_inv, A, Bx, bias):
    """Reference math on host (general per-cluster sigma_inv). Correct, not fast."""
    z2 = z[:, 0, :].astype(np.float64)
    S = np.einsum('kde,kfe->kdf', sigma_inv.astype(np.float64),
                  sigma_inv.astype(np.float64))
    d = mu.astype(np.float64)[None, :, :] - z2[:, None, :]
    d2 = np.einsum('bkd,kdf,bkf->bk', d, S, d, optimize=True)
    d2 = np.maximum(d2, MIN_CLAMP)
    lg = -d2
    lg -= lg.max(axis=1, keepdims=True)
    psi = np.exp(lg)
    psi /= psi.sum(axis=1, keepdims=True)
    G, H = _gh_from_A(A)
    ex = u.astype(np.float64) @ Bx.T.astype(np.float64) + bias.astype(np.float64)
    preds = (np.einsum('bo,kto->bkt', y[:, -ORDER:].astype(np.float64), G)
             + ex[:, :, None] * H[None, :, :])
    y_con = np.einsum('bk,bkt->bt', psi, preds).astype(np.float32)
    return y_con


def kernel(y, z, u, mu, sigma_inv, A, Bx, bias):
    global _PROGRAM, LAST_RESULTS
    y = np.asarray(y, dtype=np.float32)
    z = np.asarray(z, dtype=np.float32)
    u = np.asarray(u, dtype=np.float32)
    mu = np.asarray(mu, dtype=np.float32)
    sigma_inv = np.asarray(sigma_inv, dtype=np.float32)
    A = np.asarray(A, dtype=np.float32)
    Bx = np.asarray(Bx, dtype=np.float32)
    bias = np.asarray(bias, dtype=np.float32)

    x_recon = np.zeros((y.shape[0], IN_LEN), dtype=np.float32)

    if not np.array_equal(sigma_inv, np.broadcast_to(sigma_inv[0:1], sigma_inv.shape)):
        y_con = _host_fallback(y, z, u, mu, sigma_inv, A, Bx, bias)
        return (y_con[:, None, :], x_recon)

    in_maps = _prep_in_maps(y, z, u, mu, sigma_inv, A, Bx, bias)

    if _PROGRAM is None:
        _PROGRAM = _build_program()

    from concourse.bass_utils import run_bass_kernel_spmd
    res = run_bass_kernel_spmd(_PROGRAM, in_maps, list(range(N_CORES)), trace=TRACE)
    LAST_RESULTS = res

    # device output is [128, (tile, t)] per core -> [BPC, T_OUT]
    parts = []
    for cidx in range(N_CORES):
        oc = res.results[cidx]["ycon"].reshape(TILE, NT, T_OUT).transpose(1, 0, 2)
        parts.append(oc.reshape(BPC, T_OUT))
    y_con = np.concatenate(parts, axis=0)
    return (y_con[:, None, :], x_recon)


# revision 42
# speedup vs baseline: 1.0186x; 1.0186x over previous
"""Trainium2 Bass kernel: MoE-routed ARIMAX forecast combine.

Computes, for full inputs (B=8192 rows):
  psi   = softmax_k(-d2),  d2[b,k] = (mu_k - z_b)^T S_k (mu_k - z_b),  S_k = sig_k sig_k^T
  preds = K-expert linear AR(8) recursion rolled out T_OUT=24 steps
  y_con = sum_k psi[b,k] * preds[b,k,:]
  x_recon = zeros

Strategy:
  * Data-parallel over batch across 8 NeuronCores (1024 rows/core); the small
    parameters are replicated.
  * The AR recursion is linear in (y8, ex), so it collapses into transfer
    matrices G[K,T,O], H[K,T] precomputed on host from A (tiny, O(K*T*O)).
  * When sigma_inv is identical across clusters (true for this model: it is a
    shared covariance), z^T S z is cluster-independent and cancels in the
    row-softmax, so d2 reduces (up to a per-row additive shift) to
    c_k - 2 z.(S mu_k), which is a [B,128]x[128,32] matmul. This is also
    ~100x closer to the exact result than the reference's own f32 rounding.
  * If sigma_inv ever differed per cluster, we fall back to a host computation
    of the same math (correct for arbitrary inputs).

Device structure per core (1024 rows = 8 tiles of 128 partitions), processed
as two pipelined 4-tile halves:
  - Input DMAs split across both HWDGE queues (sync/scalar) + gpsimd SWDGE,
    sized/ordered for DMA latency: big contiguous per-partition runs
    (descriptor size), small first-wave operands first, 16-row u^T (the
    exogenous bias folds into the combine matmul as psi @ (H*bias), which
    also removes the ones-row that made the u^T DMA 17 rows/slow).
  - PE per tile: d2' = -2 z.(S mu_k) and ex = u@Bx^T as independent
    single-group matmuls into two separate PSUM banks (the softmax DVE
    reads would otherwise serialize behind the ex matmuls; and a start=True
    marks its whole 2KB bank pending-zero, so cross-matmul PSUM
    accumulation in a shared bank is avoided entirely).
  - Softmax over k batched across the half's 4 tiles with strided APs:
    +c_k (host-replicated const) -> rowmin -> sub -> exp(ACT) -> rowsum ->
    1/s -> px = e*ex  (psi deferred: 1/s is applied to the final y).
  - PE: 2 transposes of [psi|px] tile pairs; per-tile fused combine matmul
    with block-diagonal [G2 0; 0 H] -> [P | yh]; consecutive tiles use
    different PE row-groups and run concurrently, so each tile's output
    gets its own PSUM bank (concurrent row-groups writing one PSUM bank
    crash the exec unit).
  - DVE per half: y = (sum_o P[:,t,o]*y8[:,o] + yh) * r, with P t-major so
    the o-reduction reads contiguously; output DMA per half.
"""

import numpy as np

B = 8192
K = 32
D = 128
ORDER = 8
T_OUT = 24
E = 16
IN_LEN = 96
N_CORES = 8
BPC = B // N_CORES       # rows per core
TILE = 128
NT = BPC // TILE         # tiles per core
MIN_CLAMP = 0.0
OT = ORDER * T_OUT       # 192
OT1 = (ORDER + 1) * T_OUT  # 216

# packed const layout (columns of the [128, 440] const tensor); the first 96
# columns are the only ones needed by the psa matmul + softmax stages
C_W2, C_RHSA, C_CREP, C_ID, C_G2H = 0, 32, 64, 96, 224
CW = 440

_PROGRAM = None

# test.py can flip these to get a profile out of the run
TRACE = False
LAST_RESULTS = None


def _build_program():
    import concourse.bacc as bacc
    import concourse.mybir as mybir
    import concourse.tile as tile
    from contextlib import ExitStack

    F32 = mybir.dt.float32
    AX = mybir.AxisListType
    OP = mybir.AluOpType
    AF = mybir.ActivationFunctionType

    nc = bacc.Bacc("TRN2", target_bir_lowering=False, debug=False)

    zT = nc.dram_tensor("zT", [D, BPC], F32, kind="ExternalInput")
    uTa = nc.dram_tensor("uTa", [E, BPC], F32, kind="ExternalInput")
    y8d = nc.dram_tensor("y8", [TILE, NT * ORDER], F32, kind="ExternalInput")
    cst = nc.dram_tensor("cst", [TILE, CW], F32, kind="ExternalInput")
    ycon = nc.dram_tensor("ycon", [TILE, NT * T_OUT], F32, kind="ExternalOutput")

    with tile.TileContext(nc) as tc, ExitStack() as ctx:
        sb = ctx.enter_context(tc.tile_pool(name="sb", bufs=1))
        wk = ctx.enter_context(tc.tile_pool(name="wk", bufs=2))
        pst_pool = ctx.enter_context(tc.tile_pool(name="pst", bufs=1, space="PSUM"))
        psa_pool = ctx.enter_context(tc.tile_pool(name="psa", bufs=1, space="PSUM"))
        pspy_pool = ctx.enter_context(tc.tile_pool(name="pspy", bufs=1, space="PSUM"))

        # DMAs split across both HWDGE queues (sync, scalar) + gpsimd SWDGE,
        # ordered so the first half's operands land first.
        cst_s = sb.tile([TILE, CW], F32)
        zT_s = sb.tile([D, BPC], F32)
        uTa_s = sb.tile([E, BPC], F32)
        y8_s = sb.tile([TILE, NT * ORDER], F32)
        HB = BPC // 2
        nc.sync.dma_start(zT_s[:, 0:HB], zT.ap()[:, 0:HB])
        nc.scalar.dma_start(cst_s[:, 0:C_ID], cst.ap()[:, 0:C_ID])
        nc.sync.dma_start(zT_s[:, HB:BPC], zT.ap()[:, HB:BPC])
        nc.scalar.dma_start(uTa_s[:], uTa.ap())
        nc.scalar.dma_start(cst_s[:, C_ID:CW], cst.ap()[:, C_ID:CW])
        nc.gpsimd.dma_start(y8_s[:], y8d.ap())

        w2m2 = cst_s[:, C_W2:C_W2 + K]
        rhsa = cst_s[0:E, C_RHSA:C_RHSA + K]
        ident = cst_s[:, C_ID:C_ID + TILE]
        crep = cst_s[:, C_CREP:C_CREP + K]

        # psd holds -2 z.(S mu), pse holds ex -- separate PSUM banks so the
        # softmax's DVE reads of psd aren't serialized behind the uTa matmuls
        # (Tile's bank tracker orders same-bank PE-writes vs DVE-reads).
        # Every matmul is a self-contained start+stop group (a start=True
        # marks its whole 2KB bank pending-zero, so cross-matmul accumulation
        # in a shared bank only works strictly interleaved -- avoided; the c_k
        # bias is added on DVE from a host-replicated constant instead)
        # psd split per half into separate banks: otherwise half1's z matmuls
        # and half0's softmax reads share a bank and the tracker serializes
        # them, forcing all z matmuls ahead of the first softmax op
        psd0 = psa_pool.tile([TILE, 4 * K], F32, tag="psd0")
        psd1 = psa_pool.tile([TILE, 4 * K], F32, tag="psd1")
        psd01 = [psd0, psd1]
        pse = psa_pool.tile([TILE, NT * K], F32, tag="pse")
        pspy = pspy_pool.tile([TILE, 4 * 512], F32)

        for h in range(2):
            # z matmuls first: the softmax min/exp chain depends only on them
            # (ex enters later via px), so it can overlap the uTa matmuls
            psd = psd01[h]
            for i in range(4 * h, 4 * h + 4):
                q = i - 4 * h
                nc.tensor.matmul(psd[:, 32 * q:32 * q + K],
                                 zT_s[:, i * TILE:(i + 1) * TILE], w2m2,
                                 start=True, stop=True)
            for i in range(4 * h, 4 * h + 4):
                nc.tensor.matmul(pse[:, 32 * i:32 * i + K],
                                 uTa_s[:, i * TILE:(i + 1) * TILE],
                                 rhsa, start=True, stop=True)

            d2v = psd[:].rearrange("p (n w) -> p n w", w=K)
            exv = pse[:, 128 * h:128 * (h + 1)].rearrange("p (n w) -> p n w", w=K)

            # softmax over k, batched across the half's 4 tiles
            t4 = wk.tile([TILE, 4 * K], F32, tag="t4")
            nc.vector.tensor_tensor(
                t4[:].rearrange("p (n w) -> p n w", w=K), d2v,
                crep.unsqueeze(1).broadcast_to([TILE, 4, K]), op=OP.add)
            t4v = t4[:].rearrange("p (n w) -> p n w", w=K)
            min4 = wk.tile([TILE, 4], F32, tag="min4")
            nc.vector.tensor_reduce(min4[:], t4v, axis=AX.X, op=OP.min)
            dn = wk.tile([TILE, 4 * K], F32, tag="dn")
            nc.vector.tensor_tensor(
                dn[:].rearrange("p (n w) -> p n w", w=K), t4v,
                min4[:].unsqueeze(2).broadcast_to([TILE, 4, K]), op=OP.subtract)
            # epx: per tile [e(32) | px(32)]
            epx = wk.tile([TILE, 4 * 2 * K], F32, tag="epx")
            ev = epx[:].rearrange("p (n w) -> p n w", w=2 * K)[:, :, 0:K]
            pxv = epx[:].rearrange("p (n w) -> p n w", w=2 * K)[:, :, K:2 * K]
            nc.scalar.activation(ev, dn[:].rearrange("p (n w) -> p n w", w=K),
                                 AF.Exp, scale=-1.0)
            s4 = wk.tile([TILE, 4], F32, tag="s4")
            nc.vector.tensor_reduce(s4[:], ev, axis=AX.X, op=OP.add)
            r4 = wk.tile([TILE, 4], F32, tag="r4")
            nc.vector.reciprocal(r4[:], s4[:])
            nc.vector.tensor_tensor(pxv, ev, exv, op=OP.mult)

            # transpose tile pairs: [128b, psi_i|px_i|psi_j|px_j] -> [128, 128b]
            epxT = wk.tile([TILE, 2 * TILE], F32, tag="epxT")
            for jl in range(2):
                pst = pst_pool.tile([TILE, TILE], F32, tag="pst")
                nc.tensor.transpose(pst[:], epx[:, jl * TILE:(jl + 1) * TILE], ident)
                nc.scalar.copy(epxT[:, jl * TILE:(jl + 1) * TILE], pst[:])

            # per-tile fused combine matmul: [P | yh] = [psi | px] @ [G2 0; 0 H]
            # consecutive tiles use different PE row-groups (base 0/64) and run
            # concurrently -> each gets its own PSUM bank; the second half
            # reuses the banks (WAR dep on this half's combine reads orders it)
            for q in range(4):
                i = 4 * h + q
                lhsT = epxT[64 * (i % 2):64 * (i % 2) + 64,
                            (q // 2) * TILE:(q // 2) * TILE + TILE]
                rhs = cst_s[64 * (i % 2):64 * (i % 2) + 64, C_G2H:C_G2H + OT1]
                nc.tensor.matmul(pspy[:, 512 * q:512 * q + OT1], lhsT, rhs,
                                 start=True, stop=True)

            # combine: y = (sum_o P[:,t,o]*y8[:,o] + yh) * r   (t-major P layout)
            for q0, nq in [(0, 4)]:
              pv = pspy[:].rearrange("p (n w) -> p n w", w=512)[:, q0:q0 + nq, 0:OT] \
                  .rearrange("p n (t o) -> p n t o", o=ORDER)
              y8v = y8_s[:].rearrange("p (n o) -> p n o", o=ORDER) \
                  [:, 4 * h + q0:4 * h + q0 + nq, :] \
                  .unsqueeze(2).broadcast_to([TILE, nq, T_OUT, ORDER])
              m4 = wk.tile([TILE, nq * OT], F32, tag="m4")
              nc.vector.tensor_tensor(
                  m4[:].rearrange("p (n t o) -> p n t o", o=ORDER, t=T_OUT), pv, y8v,
                  op=OP.mult)
              yg4 = wk.tile([TILE, nq * T_OUT], F32, tag="yg4")
              nc.vector.tensor_reduce(
                  yg4[:].rearrange("p (n t) -> p n t", t=T_OUT),
                  m4[:].rearrange("p (n t o) -> p n t o", o=ORDER, t=T_OUT),
                  axis=AX.X, op=OP.add)
              yhv = pspy[:].rearrange("p (n w) -> p n w", w=512)[:, q0:q0 + nq, OT:OT1]
              ya4 = wk.tile([TILE, nq * T_OUT], F32, tag="ya4")
              nc.vector.tensor_tensor(
                  ya4[:].rearrange("p (n t) -> p n t", t=T_OUT),
                  yg4[:].rearrange("p (n t) -> p n t", t=T_OUT), yhv, op=OP.add)
              rv = r4[:, q0:q0 + nq].unsqueeze(2).broadcast_to([TILE, nq, T_OUT])
              yo = wk.tile([TILE, nq * T_OUT], F32, tag="yo")
              nc.vector.tensor_tensor(
                  yo[:].rearrange("p (n t) -> p n t", t=T_OUT),
                  ya4[:].rearrange("p (n t) -> p n t", t=T_OUT), rv, op=OP.mult)
              eng = nc.sync if (h == 0 or q0 == 0) else nc.scalar
              c0 = (4 * h + q0) * T_OUT
              eng.dma_start(ycon.ap()[:, c0:c0 + nq * T_OUT], yo[:])

    nc.finalize()
    return nc


def _gh_from_A(A):
    """Transfer matrices of the linear AR recursion: preds[t] = G[k,t,:]@y8 + H[k,t]*ex."""
    A64 = A.astype(np.float64)
    G = np.zeros((K, T_OUT, ORDER))
    H = np.zeros((K, T_OUT))
    wy = np.broadcast_to(np.eye(ORDER), (K, ORDER, ORDER)).copy()
    we = np.zeros((K, ORDER))
    for t in range(T_OUT):
        gy = np.einsum('ko,koj->kj', A64, wy)
        ge = np.einsum('ko,ko->k', A64, we) + 1.0
        G[:, t, :] = gy
        H[:, t] = ge
        wy = np.concatenate([wy[:, 1:, :], gy[:, None, :]], axis=1)
        we = np.concatenate([we[:, 1:], ge[:, None]], axis=1)
    return G, H


def _prep_in_maps(y, z, u, mu, sigma_inv, A, Bx, bias):
    """Host parameter preprocessing + data layout; returns per-core input maps."""
    s0 = sigma_inv[0].astype(np.float64)
    S = s0 @ s0.T
    W = S @ mu.astype(np.float64).T                    # [D, K] = S mu_k
    c = np.einsum('kd,dk->k', mu.astype(np.float64), W)
    G, H = _gh_from_A(A)

    cst = np.zeros((TILE, CW), dtype=np.float32)
    cst[:, C_W2:C_W2 + K] = (-2.0 * W).astype(np.float32)
    cst[0:E, C_RHSA:C_RHSA + K] = Bx.T
    cst[:, C_CREP:C_CREP + K] = c.astype(np.float32)[None, :]
    cst[:, C_ID:C_ID + TILE] = np.eye(TILE, dtype=np.float32)
    # psi-block rhs [G2 | H*bias]: the exogenous bias is b-independent, so its
    # preds contribution psi @ (H*bias) folds into the psi-side combine matmul
    # (this removes the ones-row from uTa -> a 16-row DMA, 16-way SDMA split)
    g2h = np.zeros((K, OT1), dtype=np.float32)
    g2h[:, 0:OT] = G.reshape(K, OT)          # t-major: [k, t*ORDER + o]
    g2h[:, OT:OT1] = H * bias.astype(np.float64)[:, None]
    cst[0:K, C_G2H:C_G2H + OT1] = g2h
    cst[K:2 * K, C_G2H + OT:C_G2H + OT1] = H
    cst[64:64 + K, C_G2H:C_G2H + OT1] = g2h           # copy for base-64 row group
    cst[64 + K:64 + 2 * K, C_G2H + OT:C_G2H + OT1] = H

    zT = np.ascontiguousarray(z[:, 0, :].T)            # [D, B]
    uTa = np.ascontiguousarray(u.T)                    # [E, B]
    y8 = np.ascontiguousarray(y[:, IN_LEN - ORDER:])   # [B, 8]

    in_maps = []
    for cidx in range(N_CORES):
        lo, hi = cidx * BPC, (cidx + 1) * BPC
        y8c = y8[lo:hi].reshape(NT, TILE, ORDER).transpose(1, 0, 2)
        in_maps.append({
            "zT": np.ascontiguousarray(zT[:, lo:hi]),
            "uTa": np.ascontiguousarray(uTa[:, lo:hi]),
            "y8": np.ascontiguousarray(y8c.reshape(TILE, NT * ORDER)),
            "cst": cst,
        })
    return in_maps


def _host_fallback(y, z, u, mu, sigma_inv, A, Bx, bias):
    """Reference math on host (general per-cluster sigma_inv). Correct, not fast."""
    z2 = z[:, 0, :].astype(np.float64)
    S = np.einsum('kde,kfe->kdf', sigma_inv.astype(np.float64),
                  sigma_inv.astype(np.float64))
    d = mu.astype(np.float64)[None, :, :] - z2[:, None, :]
    d2 = np.einsum('bkd,kdf,bkf->bk', d, S, d, optimize=True)
    d2 = np.maximum(d2, MIN_CLAMP)
    lg = -d2
    lg -= lg.max(axis=1, keepdims=True)
    psi = np.exp(lg)
    psi /= psi.sum(axis=1, keepdims=True)
    G, H = _gh_from_A(A)
    ex = u.astype(np.float64) @ Bx.T.astype(np.float64) + bias.astype(np.float64)
    preds = (np.einsum('bo,kto->bkt', y[:, -ORDER:].astype(np.float64), G)
             + ex[:, :, None] * H[None, :, :])
    y_con = np.einsum('bk,bkt->bt', psi, preds).astype(np.float32)
    return y_con


def kernel(y, z, u, mu, sigma_inv, A, Bx, bias):
    global _PROGRAM, LAST_RESULTS
    y = np.asarray(y, dtype=np.float32)
    z = np.asarray(z, dtype=np.float32)
    u = np.asarray(u, dtype=np.float32)
    mu = np.asarray(mu, dtype=np.float32)
    sigma_inv = np.asarray(sigma_inv, dtype=np.float32)
    A = np.asarray(A, dtype=np.float32)
    Bx = np.asarray(Bx, dtype=np.float32)
    bias = np.asarray(bias, dtype=np.float32)

    x_recon = np.zeros((y.shape[0], IN_LEN), dtype=np.float32)

    if not np.array_equal(sigma_inv, np.broadcast_to(sigma_inv[0:1], sigma_inv.shape)):
        y_con = _host_fallback(y, z, u, mu, sigma_inv, A, Bx, bias)
        return (y_con[:, None, :], x_recon)

    in_maps = _prep_in_maps(y, z, u, mu, sigma_inv, A, Bx, bias)

    if _PROGRAM is None:
        _PROGRAM = _build_program()

    from concourse.bass_utils import run_bass_kernel_spmd
    res = run_bass_kernel_spmd(_PROGRAM, in_maps, list(range(N_CORES)), trace=TRACE)
    LAST_RESULTS = res

    # device output is [128, (tile, t)] per core -> [BPC, T_OUT]
    parts = []
    for cidx in range(N_CORES):
        oc = res.results[cidx]["ycon"].reshape(TILE, NT, T_OUT).transpose(1, 0, 2)
        parts.append(oc.reshape(BPC, T_OUT))
    y_con = np.concatenate(parts, axis=0)
    return (y_con[:, None, :], x_recon)
